# revision 9
# baseline (speedup 1.0000x reference)
"""AFNO2D (channel-first) Trainium2 kernel.

out = x + irfft2( softshrink(mlp2(leaky(mlp1(rfft2(x))))) * rfft2(x) )
with block-diagonal complex MLPs over 8 channel blocks of 96.

Sharding: block-parallel — core k owns spectral block k (96 channels), fully
independent, zero collectives. All DFTs are dense bf16 matmuls on the
TensorEngine with ortho scaling folded into host-precomputed twiddle
matrices. Residual add in f32 on device.

Hardcoded shapes: x [4,768,128,128] f32, w1/w2 [2,8,96,96], b1/b2 [2,8,96].
"""

import os
import numpy as np
import ml_dtypes

B, C, H, W = 4, 768, 128, 128
NBLK, BS = 8, 96          # spectral blocks, channels per core
WF = 65                   # rfft size along W
LAM = 0.01                # softshrink threshold
NS = 0.1                  # leaky relu negative slope

BF16 = ml_dtypes.bfloat16

LAST_RESULT = {}          # diagnostics (exec_time_ns) for the test harness


def _twiddles():
    n = 128
    wv = np.arange(n)[:, None].astype(np.float64)
    jv = np.arange(n)[None, :].astype(np.float64)
    ang = 2.0 * np.pi * wv * jv / n  # [128,128]

    # S1 moving operand [w, 128pack]: cols 0..64 cos/n ; cols 65..127 -sin/n (wf 1..63)
    fw = np.zeros((n, n), np.float64)
    fw[:, :WF] = np.cos(ang[:, :WF]) / n
    fw[:, WF:] = -np.sin(ang[:, 1:64]) / n

    s2c = np.cos(ang)       # [h, hf] symmetric
    s2s = np.sin(ang)
    s2sn = -s2s

    # final irfft_W moving operands
    alpha = np.full((WF, 1), 2.0)
    alpha[0, 0] = 1.0
    alpha[64, 0] = 1.0
    fic = alpha * np.cos(2.0 * np.pi * np.arange(WF)[:, None] * np.arange(n)[None, :] / n) / n
    fis = -2.0 * np.sin(2.0 * np.pi * np.arange(1, 64)[:, None] * np.arange(n)[None, :] / n) / n

    ident = np.eye(n)
    return (fw.astype(BF16), s2c.astype(BF16), s2s.astype(BF16),
            s2sn.astype(BF16), fic.astype(BF16), fis.astype(BF16),
            ident.astype(BF16))


def _build():
    import concourse.mybir as mybir
    import concourse.tile as tile
    from concourse import bacc

    dt = mybir.dt
    AF = mybir.ActivationFunctionType
    ALU = mybir.AluOpType

    nc = bacc.Bacc("TRN2", target_bir_lowering=False, debug=False)

    xt = nc.declare_dram_parameter("xt", [B, W, BS, H], dt.bfloat16, isOutput=False)
    xres = nc.declare_dram_parameter("xres", [B, BS, H, W], dt.float32, isOutput=False)
    out = nc.declare_dram_parameter("out", [B, BS, H, W], dt.float32, isOutput=True)

    fw_d = nc.declare_dram_parameter("fw", [128, 128], dt.bfloat16, isOutput=False)
    s2c_d = nc.declare_dram_parameter("s2c", [128, 128], dt.bfloat16, isOutput=False)
    s2s_d = nc.declare_dram_parameter("s2s", [128, 128], dt.bfloat16, isOutput=False)
    s2sn_d = nc.declare_dram_parameter("s2sn", [128, 128], dt.bfloat16, isOutput=False)
    fic_d = nc.declare_dram_parameter("fic", [WF, 128], dt.bfloat16, isOutput=False)
    fis_d = nc.declare_dram_parameter("fis", [63, 128], dt.bfloat16, isOutput=False)
    id_d = nc.declare_dram_parameter("ident", [128, 128], dt.bfloat16, isOutput=False)

    dbg = bool(int(os.environ.get("AFNO_DEBUG", "0")))
    if dbg:
        d_u1 = nc.declare_dram_parameter("d_u1", [128, BS, 128], dt.bfloat16, isOutput=True)
        d_ub = nc.declare_dram_parameter("d_ub", [128, BS, 130], dt.bfloat16, isOutput=True)
        d_utr = nc.declare_dram_parameter("d_utr", [BS, 2 * WF * 128], dt.bfloat16, isOutput=True)
        d_yt = nc.declare_dram_parameter("d_yt", [128, BS, 130], dt.bfloat16, isOutput=True)
    wds = {nm: nc.declare_dram_parameter(nm, [BS, BS], dt.bfloat16, isOutput=False)
           for nm in ("w1r", "w1i", "w1in", "w2r", "w2i", "w2in")}
    bds = {nm: nc.declare_dram_parameter(nm, [BS, 1], dt.float32, isOutput=False)
           for nm in ("b1r", "b1i", "b2r", "b2i")}

    with tile.TileContext(nc) as tc:
        with (
            tc.tile_pool(name="consts", bufs=1) as consts,
            tc.tile_pool(name="xts", bufs=3) as xts_p,
            tc.tile_pool(name="u1", bufs=1) as u1_p,
            tc.tile_pool(name="big", bufs=2) as big_p,
            tc.tile_pool(name="utr", bufs=1) as utr_p,
            tc.tile_pool(name="chunk", bufs=4) as chunk_p,
            tc.tile_pool(name="gtmp", bufs=4) as gtmp_p,
            tc.tile_pool(name="vb", bufs=4) as vb_p,
            tc.tile_pool(name="vtb", bufs=3) as vtb_p,
            tc.tile_pool(name="iot", bufs=3) as iot_p,
            tc.tile_pool(name="pA", bufs=4, space="PSUM") as pA,
            tc.tile_pool(name="pM1", bufs=2, space="PSUM") as pM1,
            tc.tile_pool(name="pM2", bufs=2, space="PSUM") as pM2,
        ):
            # ---------------- constants ----------------
            def cload(dparam, shape, dtype, tag):
                t = consts.tile(shape, dtype, tag=tag, name=tag)
                nc.sync.dma_start(out=t[:], in_=dparam[:, :])
                return t

            fw = cload(fw_d, [128, 128], dt.bfloat16, "fw")
            s2c = cload(s2c_d, [128, 128], dt.bfloat16, "s2c")
            s2s = cload(s2s_d, [128, 128], dt.bfloat16, "s2s")
            s2sn = cload(s2sn_d, [128, 128], dt.bfloat16, "s2sn")
            fic = cload(fic_d, [WF, 128], dt.bfloat16, "fic")
            fis = cload(fis_d, [63, 128], dt.bfloat16, "fis")
            ident = cload(id_d, [128, 128], dt.bfloat16, "ident")
            wt = {nm: cload(wds[nm], [BS, BS], dt.bfloat16, nm) for nm in wds}
            bt = {nm: cload(bds[nm], [BS, 1], dt.float32, nm) for nm in bds}

            cgroups = [list(range(s, min(s + 9, BS))) for s in range(0, BS, 9)]

            for b in range(B):
                # ---------- load x transposed (two halves) ----------
                xh = []
                for hh in range(2):
                    t = xts_p.tile([128, 48, 128], dt.bfloat16, tag="xts", name="xts")
                    nc.sync.dma_start(out=t[:], in_=xt[b, :, hh * 48:(hh + 1) * 48, :])
                    xh.append(t)

                # ---------- S1: rfft along W (data-stationary) ----------
                u1 = u1_p.tile([128, BS, 128], dt.bfloat16, tag="u1")
                for g in range(BS // 3):
                    ps = pA.tile([128, 384], dt.float32, tag="pA")
                    for k in range(3):
                        c = 3 * g + k
                        lhs = xh[c // 48][:, c % 48, :]
                        nc.tensor.matmul(ps[:, k * 128:(k + 1) * 128], lhs, fw,
                                         start=(k == 0), stop=(k == 2),
                                         skip_group_check=True)
                    nc.any.tensor_copy(
                        u1[:, 3 * g:3 * g + 3, :],
                        ps[:, :].rearrange("p (c f) -> p c f", c=3))

                if dbg and b == 0:
                    nc.sync.dma_start(out=d_u1[:, :, :], in_=u1[:, :, :])
                # ---------- S2: DFT along H ----------
                ub = big_p.tile([128, BS, 130], dt.bfloat16, tag="big")
                for grp in cgroups:
                    tiles3 = [grp[i:i + 3] for i in range(0, len(grp), 3)]
                    pss = [pA.tile([128, 390], dt.float32, tag="pA", name="pss2") for _ in tiles3]
                    for ps, t3 in zip(pss, tiles3):           # cos -> Ur[0:65]
                        for k, c in enumerate(t3):
                            nc.tensor.matmul(ps[:, k * 130:k * 130 + 65],
                                             s2c, u1[:, c, 0:65],
                                             start=(k == 0), stop=False,
                                             skip_group_check=True)
                    for ps, t3 in zip(pss, tiles3):           # sin -> Ur[1:64]
                        for k, c in enumerate(t3):
                            nc.tensor.matmul(ps[:, k * 130 + 1:k * 130 + 64],
                                             s2s, u1[:, c, 65:128],
                                             start=False, stop=False,
                                             skip_group_check=True)
                    for ps, t3 in zip(pss, tiles3):           # -sin -> Ui[0:65]
                        for k, c in enumerate(t3):
                            nc.tensor.matmul(ps[:, k * 130 + 65:k * 130 + 130],
                                             s2sn, u1[:, c, 0:65],
                                             start=False, stop=False,
                                             skip_group_check=True)
                    for ps, t3 in zip(pss, tiles3):           # cos -> Ui[1:64]
                        for k, c in enumerate(t3):
                            nc.tensor.matmul(ps[:, k * 130 + 66:k * 130 + 129],
                                             s2c, u1[:, c, 65:128],
                                             start=False, stop=(k == len(t3) - 1),
                                             skip_group_check=True)
                    for ps, t3 in zip(pss, tiles3):
                        nc.any.tensor_copy(
                            ub[:, t3[0]:t3[0] + len(t3), :],
                            ps[:, 0:130 * len(t3)].rearrange(
                                "p (c f) -> p c f", c=len(t3)))

                # ---------- preT: [hf,(c,130)] -> utr [c, (r|i, wf, hf)] ----------
                utr = utr_p.tile([BS, 2 * WF * 128], dt.bfloat16, tag="utr")
                for half in range(2):
                    base = half * (WF * 128)
                    for j0 in range(0, WF, 4):
                        jj = list(range(j0, min(j0 + 4, WF)))
                        ps = pA.tile([128, 512], dt.bfloat16, tag="pA")
                        for k, j in enumerate(jj):
                            nc.tensor.transpose(ps[0:BS, k * 128:(k + 1) * 128],
                                                ub[:, :, half * 65 + j], ident)
                        nc.any.tensor_copy(
                            utr[:, base + j0 * 128: base + (j0 + len(jj)) * 128],
                            ps[0:BS, 0:len(jj) * 128])

                if dbg and b == 0:
                    nc.sync.dma_start(out=d_ub[:, :, :], in_=ub[:, :, :])
                    nc.sync.dma_start(out=d_utr[:, :], in_=utr[:, :])
                # ---------- mix1 -> leaky -> mix2 -> shrink -> gate -> backT ----
                yt = big_p.tile([128, BS, 130], dt.bfloat16, tag="big")
                nch = (WF * 128) // 256
                chunks = [(ci * 256, 256) for ci in range(nch)] + \
                         [(nch * 256, WF * 128 - nch * 256)]
                for (off, sz) in chunks:
                    ur = utr[:, off:off + sz]
                    ui = utr[:, WF * 128 + off:WF * 128 + off + sz]
                    p1 = pM1.tile([BS, 512], dt.float32, tag="pM1")
                    nc.tensor.matmul(p1[:, 0:sz], wt["w1r"], ur,
                                     start=True, stop=False,
                                     skip_group_check=True)
                    nc.tensor.matmul(p1[:, 256:256 + sz], wt["w1r"], ui,
                                     start=False, stop=False,
                                     skip_group_check=True)
                    nc.tensor.matmul(p1[:, 0:sz], wt["w1in"], ui,
                                     start=False, stop=False,
                                     skip_group_check=True)
                    nc.tensor.matmul(p1[:, 256:256 + sz], wt["w1i"], ur,
                                     start=False, stop=True,
                                     skip_group_check=True)
                    o1 = chunk_p.tile([BS, 512], dt.bfloat16, tag="o1")
                    nc.scalar.activation(o1[:, 0:sz], p1[:, 0:sz], AF.Prelu,
                                         bias=bt["b1r"], scale=1.0, alpha=NS)
                    nc.scalar.activation(o1[:, 256:256 + sz], p1[:, 256:256 + sz],
                                         AF.Prelu, bias=bt["b1i"], scale=1.0, alpha=NS)
                    p2 = pM2.tile([BS, 512], dt.float32, tag="pM2")
                    nc.tensor.matmul(p2[:, 0:sz], wt["w2r"], o1[:, 0:sz],
                                     start=True, stop=False,
                                     skip_group_check=True)
                    nc.tensor.matmul(p2[:, 256:256 + sz], wt["w2r"], o1[:, 256:256 + sz],
                                     start=False, stop=False,
                                     skip_group_check=True)
                    nc.tensor.matmul(p2[:, 0:sz], wt["w2in"], o1[:, 256:256 + sz],
                                     start=False, stop=False,
                                     skip_group_check=True)
                    nc.tensor.matmul(p2[:, 256:256 + sz], wt["w2i"], o1[:, 0:sz],
                                     start=False, stop=True,
                                     skip_group_check=True)
                    z = chunk_p.tile([BS, 512], dt.bfloat16, tag="z")
                    nc.scalar.activation(z[:, 0:sz], p2[:, 0:sz], AF.Identity,
                                         bias=bt["b2r"], scale=1.0)
                    nc.scalar.activation(z[:, 256:256 + sz], p2[:, 256:256 + sz],
                                         AF.Identity, bias=bt["b2i"], scale=1.0)
                    cl = chunk_p.tile([BS, 512], dt.bfloat16, tag="cl")
                    nc.vector.tensor_scalar(cl[:, :], z[:, :], -LAM, LAM,
                                            ALU.max, ALU.min)
                    sh = chunk_p.tile([BS, 512], dt.bfloat16, tag="sh")
                    nc.vector.tensor_sub(sh[:, :], z[:, :], cl[:, :])
                    # gate: y = s * U (complex elementwise)
                    t1 = gtmp_p.tile([BS, 256], dt.bfloat16, tag="t1")
                    t2 = gtmp_p.tile([BS, 256], dt.bfloat16, tag="t2")
                    yg = gtmp_p.tile([BS, 512], dt.bfloat16, tag="yg")
                    nc.vector.tensor_mul(t1[:, 0:sz], sh[:, 0:sz], ur)
                    nc.vector.tensor_mul(t2[:, 0:sz], sh[:, 256:256 + sz], ui)
                    nc.vector.tensor_sub(yg[:, 0:sz], t1[:, 0:sz], t2[:, 0:sz])
                    nc.vector.tensor_mul(t1[:, 0:sz], sh[:, 0:sz], ui)
                    nc.vector.tensor_mul(t2[:, 0:sz], sh[:, 256:256 + sz], ur)
                    nc.vector.tensor_add(yg[:, 256:256 + sz], t1[:, 0:sz], t2[:, 0:sz])
                    # backT into yt [hf, (c, 130)]
                    nsl = sz // 128
                    ps = pA.tile([128, 512], dt.bfloat16, tag="pA")
                    for sl in range(nsl):
                        nc.tensor.transpose(ps[:, sl * 96:(sl + 1) * 96],
                                            yg[:, sl * 128:(sl + 1) * 128],
                                            ident[0:BS, 0:BS])
                        nc.tensor.transpose(ps[:, (nsl + sl) * 96:(nsl + sl + 1) * 96],
                                            yg[:, 256 + sl * 128:256 + (sl + 1) * 128],
                                            ident[0:BS, 0:BS])
                    j0 = off // 128
                    yr_dst = yt[:, :, j0:j0 + nsl].rearrange("p c j -> p j c")
                    yi_dst = yt[:, :, 65 + j0:65 + j0 + nsl].rearrange("p c j -> p j c")
                    nc.any.tensor_copy(
                        yr_dst, ps[:, 0:nsl * 96].rearrange("p (j c) -> p j c", j=nsl))
                    nc.any.tensor_copy(
                        yi_dst, ps[:, nsl * 96:2 * nsl * 96].rearrange(
                            "p (j c) -> p j c", j=nsl))

                if dbg and b == 0:
                    nc.sync.dma_start(out=d_yt[:, :, :], in_=yt[:, :, :])
                # ---------- iDFT along H + final irfft_W + residual ----------
                for grp in cgroups:
                    tiles3 = [grp[i:i + 3] for i in range(0, len(grp), 3)]
                    pss = [pA.tile([128, 384], dt.float32, tag="pA", name="pssv") for _ in tiles3]
                    for ps, t3 in zip(pss, tiles3):           # cos.Yr -> Vr
                        for k, c in enumerate(t3):
                            nc.tensor.matmul(ps[:, k * 128:k * 128 + 65],
                                             s2c, yt[:, c, 0:65],
                                             start=(k == 0), stop=False,
                                             skip_group_check=True)
                    for ps, t3 in zip(pss, tiles3):           # -sin.Yi -> Vr
                        for k, c in enumerate(t3):
                            nc.tensor.matmul(ps[:, k * 128:k * 128 + 65],
                                             s2sn, yt[:, c, 65:130],
                                             start=False, stop=False,
                                             skip_group_check=True)
                    for ps, t3 in zip(pss, tiles3):           # sin.Yr -> Vi[1:64]
                        for k, c in enumerate(t3):
                            nc.tensor.matmul(ps[:, k * 128 + 65:k * 128 + 128],
                                             s2s, yt[:, c, 1:64],
                                             start=False, stop=False,
                                             skip_group_check=True)
                    for ps, t3 in zip(pss, tiles3):           # cos.Yi -> Vi[1:64]
                        for k, c in enumerate(t3):
                            nc.tensor.matmul(ps[:, k * 128 + 65:k * 128 + 128],
                                             s2c, yt[:, c, 66:129],
                                             start=False, stop=(k == len(t3) - 1),
                                             skip_group_check=True)
                    vbs = []
                    for ps, t3 in zip(pss, tiles3):
                        vb = vb_p.tile([128, 384], dt.bfloat16, tag="vb")
                        nc.any.tensor_copy(vb[:, 0:len(t3) * 128],
                                           ps[:, 0:len(t3) * 128])
                        vbs.append(vb)
                    for vb, t3 in zip(vbs, tiles3):
                        n3 = len(t3)
                        psr = pA.tile([128, 512], dt.bfloat16, tag="pA")
                        for k in range(n3):
                            nc.tensor.transpose(psr[0:WF, k * 128:(k + 1) * 128],
                                                vb[:, k * 128:k * 128 + 65], ident)
                        psi = pA.tile([128, 512], dt.bfloat16, tag="pA")
                        for k in range(n3):
                            nc.tensor.transpose(psi[0:63, k * 128:(k + 1) * 128],
                                                vb[:, k * 128 + 65:(k + 1) * 128], ident)
                        vtr = vtb_p.tile([WF, 384], dt.bfloat16, tag="vtr")
                        nc.any.tensor_copy(vtr[:, 0:n3 * 128], psr[0:WF, 0:n3 * 128])
                        vti = vtb_p.tile([63, 384], dt.bfloat16, tag="vti")
                        nc.any.tensor_copy(vti[:, 0:n3 * 128], psi[0:63, 0:n3 * 128])
                        po = pA.tile([128, 384], dt.float32, tag="pA")
                        for k in range(n3):
                            nc.tensor.matmul(po[:, k * 128:(k + 1) * 128],
                                             vtr[:, k * 128:(k + 1) * 128], fic,
                                             start=(k == 0), stop=False,
                                             skip_group_check=True)
                            nc.tensor.matmul(po[:, k * 128:(k + 1) * 128],
                                             vti[:, k * 128:(k + 1) * 128], fis,
                                             start=False, stop=(k == n3 - 1),
                                             skip_group_check=True)
                        xr = iot_p.tile([128, 384], dt.float32, tag="xr")
                        for k, c in enumerate(t3):
                            nc.sync.dma_start(out=xr[:, k * 128:(k + 1) * 128],
                                              in_=xres[b, c, :, :])
                        ot = iot_p.tile([128, 384], dt.float32, tag="ot")
                        nc.vector.tensor_add(ot[:, 0:n3 * 128], po[:, 0:n3 * 128],
                                             xr[:, 0:n3 * 128])
                        for k, c in enumerate(t3):
                            nc.sync.dma_start(out=out[b, c, :, :],
                                              in_=ot[:, k * 128:(k + 1) * 128])

    nc.finalize()
    return nc


_BUILT = None


def _get_built():
    global _BUILT
    if _BUILT is None:
        _BUILT = _build()
    return _BUILT


def kernel(x, w1, b1, w2, b2):
    from concourse.bass_utils import run_bass_kernel_spmd

    nc = _get_built()
    fw, s2c, s2s, s2sn, fic, fis, ident = _twiddles()

    in_maps = []
    for k in range(NBLK):
        xs = x[:, k * BS:(k + 1) * BS]  # [B, 96, H, W]
        m = {
            "xt": np.ascontiguousarray(xs.transpose(0, 3, 1, 2)).astype(BF16),
            "xres": np.ascontiguousarray(xs).astype(np.float32),
            "fw": fw, "s2c": s2c, "s2s": s2s, "s2sn": s2sn,
            "fic": fic, "fis": fis, "ident": ident,
            "w1r": w1[0, k].astype(BF16), "w1i": w1[1, k].astype(BF16),
            "w1in": (-w1[1, k]).astype(BF16),
            "w2r": w2[0, k].astype(BF16), "w2i": w2[1, k].astype(BF16),
            "w2in": (-w2[1, k]).astype(BF16),
            "b1r": b1[0, k].reshape(BS, 1).astype(np.float32),
            "b1i": b1[1, k].reshape(BS, 1).astype(np.float32),
            "b2r": b2[0, k].reshape(BS, 1).astype(np.float32),
            "b2i": b2[1, k].reshape(BS, 1).astype(np.float32),
        }
        in_maps.append(m)

    trace = bool(int(os.environ.get("AFNO_TRACE", "0")))
    kw = {}
    if trace:
        import tempfile
        kw["tmpdir"] = tempfile.mkdtemp(prefix="afno_trace_")
        LAST_RESULT["trace_dir"] = kw["tmpdir"]
    res = run_bass_kernel_spmd(nc, in_maps, core_ids=list(range(NBLK)),
                               trace=trace, **kw)
    LAST_RESULT["exec_time_ns"] = res.exec_time_ns

    outp = np.empty((B, C, H, W), np.float32)
    for k in range(NBLK):
        outp[:, k * BS:(k + 1) * BS] = res.results[k]["out"]
    return outp


# revision 12
# speedup vs baseline: 1.1572x; 1.1572x over previous
"""AFNO2D (channel-first) Trainium2 kernel.

out = x + irfft2( softshrink(mlp2(leaky(mlp1(rfft2(x))))) * rfft2(x) )
with block-diagonal complex MLPs over 8 channel blocks of 96.

Sharding: block-parallel — core k owns spectral block k (96 channels), fully
independent, zero collectives. All DFTs are dense bf16 matmuls on the
TensorEngine with ortho scaling folded into host-precomputed twiddle
matrices. Biases are folded into augmented stationaries via an ones-row.
Residual add in f32 on device.

PSUM rule: matmul start=True clears has_written for the WHOLE bank, so each
PSUM tile gets exactly one start=True (its first matmul); all later matmuls
use start=False (fresh ranges overwrite, accumulation ranges add).

Hardcoded shapes: x [4,768,128,128] f32, w1/w2 [2,8,96,96], b1/b2 [2,8,96].
"""

import os
import numpy as np
import ml_dtypes

B, C, H, W = 4, 768, 128, 128
NBLK, BS = 8, 96          # spectral blocks, channels per core
WF = 65                   # rfft size along W
LAM = 0.01                # softshrink threshold
NS = 0.1                  # leaky relu negative slope

BF16 = ml_dtypes.bfloat16

LAST_RESULT = {}          # diagnostics (exec_time_ns) for the test harness


def _twiddles():
    n = 128
    wv = np.arange(n)[:, None].astype(np.float64)
    jv = np.arange(n)[None, :].astype(np.float64)
    ang = 2.0 * np.pi * wv * jv / n  # [128,128]

    # S1 moving operand [w, 130]: cols 0..64 cos/n ; cols 65..129 -sin/n
    # (imag cols 65 and 129 i.e. wf=0,64 are exactly zero)
    fw = np.zeros((n, 130), np.float64)
    fw[:, :WF] = np.cos(ang[:, :WF]) / n
    fw[:, WF + 1:WF + 64] = -np.sin(ang[:, 1:64]) / n

    s2c = np.cos(ang)       # [h, hf] symmetric
    s2s = np.sin(ang)
    s2sn = -s2s

    # final irfft_W moving operands
    alpha = np.full((WF, 1), 2.0)
    alpha[0, 0] = 1.0
    alpha[64, 0] = 1.0
    fic = alpha * np.cos(2.0 * np.pi * np.arange(WF)[:, None] * np.arange(n)[None, :] / n) / n
    fis = -2.0 * np.sin(2.0 * np.pi * np.arange(1, 64)[:, None] * np.arange(n)[None, :] / n) / n

    ident = np.eye(n)
    return (fw.astype(BF16), s2c.astype(BF16), s2s.astype(BF16),
            s2sn.astype(BF16), fic.astype(BF16), fis.astype(BF16),
            ident.astype(BF16))


def _build():
    import concourse.mybir as mybir
    import concourse.tile as tile
    from concourse import bacc

    dt = mybir.dt
    AF = mybir.ActivationFunctionType
    ALU = mybir.AluOpType

    nc = bacc.Bacc("TRN2", target_bir_lowering=False, debug=False)

    xt = nc.declare_dram_parameter("xt", [B, W, BS, H], dt.bfloat16, isOutput=False)
    xres = nc.declare_dram_parameter("xres", [B, BS, H, W], dt.float32, isOutput=False)
    out = nc.declare_dram_parameter("out", [B, BS, H, W], dt.float32, isOutput=True)

    fw_d = nc.declare_dram_parameter("fw", [128, 130], dt.bfloat16, isOutput=False)
    s2c_d = nc.declare_dram_parameter("s2c", [128, 128], dt.bfloat16, isOutput=False)
    s2s_d = nc.declare_dram_parameter("s2s", [128, 128], dt.bfloat16, isOutput=False)
    s2sn_d = nc.declare_dram_parameter("s2sn", [128, 128], dt.bfloat16, isOutput=False)
    fic_d = nc.declare_dram_parameter("fic", [WF, 128], dt.bfloat16, isOutput=False)
    fis_d = nc.declare_dram_parameter("fis", [63, 128], dt.bfloat16, isOutput=False)
    id_d = nc.declare_dram_parameter("ident", [128, 128], dt.bfloat16, isOutput=False)

    # augmented stationaries: w1ra=[w1r;b1r] etc (ones-row bias fold)
    wnames = ("w1ra", "w1ia", "w1r", "w1in", "w2ra", "w2ia", "w2r", "w2in")
    wshapes = {"w1ra": BS + 1, "w1ia": BS + 1, "w2ra": BS + 1, "w2ia": BS + 1,
               "w1r": BS, "w1in": BS, "w2r": BS, "w2in": BS}
    wds = {nm: nc.declare_dram_parameter(nm, [wshapes[nm], BS], dt.bfloat16,
                                         isOutput=False) for nm in wnames}

    dbg = bool(int(os.environ.get("AFNO_DEBUG", "0")))
    if dbg:
        d_u1 = nc.declare_dram_parameter("d_u1", [128, BS, 130], dt.bfloat16, isOutput=True)
        d_ub = nc.declare_dram_parameter("d_ub", [128, BS, 130], dt.bfloat16, isOutput=True)
        d_utr = nc.declare_dram_parameter("d_utr", [BS, 2 * WF * 128], dt.bfloat16, isOutput=True)
        d_yt = nc.declare_dram_parameter("d_yt", [128, BS, 130], dt.bfloat16, isOutput=True)

    with tile.TileContext(nc) as tc:
        with (
            tc.tile_pool(name="consts", bufs=1) as consts,
            tc.tile_pool(name="xts", bufs=3) as xts_p,
            tc.tile_pool(name="u1", bufs=1) as u1_p,
            tc.tile_pool(name="big", bufs=2) as big_p,
            tc.tile_pool(name="utr", bufs=1) as utr_p,
            tc.tile_pool(name="chunk", bufs=4) as chunk_p,
            tc.tile_pool(name="gtmp", bufs=4) as gtmp_p,
            tc.tile_pool(name="vb", bufs=4) as vb_p,
            tc.tile_pool(name="vtb", bufs=3) as vtb_p,
            tc.tile_pool(name="iot", bufs=3) as iot_p,
            tc.tile_pool(name="pA", bufs=4, space="PSUM") as pA,
            tc.tile_pool(name="pM1", bufs=2, space="PSUM") as pM1,
            tc.tile_pool(name="pM2", bufs=2, space="PSUM") as pM2,
        ):
            # ---------------- constants ----------------
            def cload(dparam, shape, dtype, tag):
                t = consts.tile(shape, dtype, tag=tag, name=tag)
                nc.sync.dma_start(out=t[:], in_=dparam[:, :])
                return t

            fw = cload(fw_d, [128, 130], dt.bfloat16, "fw")
            s2c = cload(s2c_d, [128, 128], dt.bfloat16, "s2c")
            s2s = cload(s2s_d, [128, 128], dt.bfloat16, "s2s")
            s2sn = cload(s2sn_d, [128, 128], dt.bfloat16, "s2sn")
            fic = cload(fic_d, [WF, 128], dt.bfloat16, "fic")
            fis = cload(fis_d, [63, 128], dt.bfloat16, "fis")
            ident = cload(id_d, [128, 128], dt.bfloat16, "ident")
            wt = {nm: cload(wds[nm], [wshapes[nm], BS], dt.bfloat16, nm)
                  for nm in wnames}

            for b in range(B):
                # ---------- load x transposed (two halves) ----------
                xh = []
                for hh in range(2):
                    t = xts_p.tile([128, 48, 128], dt.bfloat16, tag="xts", name="xts")
                    nc.sync.dma_start(out=t[:], in_=xt[b, :, hh * 48:(hh + 1) * 48, :])
                    xh.append(t)

                # ---------- S1: rfft along W (data-stationary) ----------
                u1 = u1_p.tile([128, BS, 130], dt.bfloat16, tag="u1")
                for g in range(BS // 3):
                    ps = pA.tile([128, 3, 130], dt.float32, tag="pA", name="ps1")
                    for k in range(3):
                        c = 3 * g + k
                        lhs = xh[c // 48][:, c % 48, :]
                        nc.tensor.matmul(ps[:, k, :], lhs, fw,
                                         start=(k == 0), stop=(k == 2),
                                         skip_group_check=True)
                    nc.any.tensor_copy(u1[:, 3 * g:3 * g + 3, :], ps[:, :, :])
                if dbg and b == 0:
                    nc.sync.dma_start(out=d_u1[:, :, :], in_=u1[:, :, :])

                # ---------- S2: DFT along H (3 channels per matmul) ----------
                ub = big_p.tile([128, BS, 130], dt.bfloat16, tag="big", name="ub")
                for g0 in range(0, BS // 3, 3):    # groups of 3 psum tiles (9 ch)
                    tl = [g0 + i for i in range(3) if g0 + i < BS // 3]
                    # psum layout [128, (half2, c3, wf65)]: all MM outs contiguous
                    pss = [pA.tile([128, 390], dt.float32, tag="pA", name="ps2")
                           for _ in tl]
                    for ps, t in zip(pss, tl):     # cos.U1r -> Ur
                        nc.tensor.matmul(ps[:, 0:195], s2c,
                                         u1[:, 3 * t:3 * t + 3, 0:65],
                                         start=True, stop=False,
                                         skip_group_check=True)
                    for ps, t in zip(pss, tl):     # sin.U1i -> Ur (acc)
                        nc.tensor.matmul(ps[:, 0:195], s2s,
                                         u1[:, 3 * t:3 * t + 3, 65:130],
                                         start=False, stop=False,
                                         skip_group_check=True)
                    for ps, t in zip(pss, tl):     # -sin.U1r -> Ui
                        nc.tensor.matmul(ps[:, 195:390], s2sn,
                                         u1[:, 3 * t:3 * t + 3, 0:65],
                                         start=False, stop=False,
                                         skip_group_check=True)
                    for ps, t in zip(pss, tl):     # cos.U1i -> Ui (acc)
                        nc.tensor.matmul(ps[:, 195:390], s2c,
                                         u1[:, 3 * t:3 * t + 3, 65:130],
                                         start=False, stop=True,
                                         skip_group_check=True)
                    for ps, t in zip(pss, tl):
                        nc.any.tensor_copy(
                            ub[:, 3 * t:3 * t + 3, :].rearrange(
                                "p c (h w) -> p c h w", h=2),
                            ps[:, :].rearrange("p (h c w) -> p c h w", h=2, c=3))

                # ---------- preT: [hf,(c,130)] -> utr [c, (r|i, wf, hf)] ------
                utr = utr_p.tile([BS + 1, 2 * WF * 128], dt.bfloat16, tag="utr")
                nc.gpsimd.memset(utr[BS:BS + 1, :], 1.0)   # ones-row (bias fold)
                for half in range(2):
                    base = half * (WF * 128)
                    for j0 in range(0, WF, 4):
                        jj = list(range(j0, min(j0 + 4, WF)))
                        ps = pA.tile([128, 512], dt.bfloat16, tag="pA", name="pst")
                        for k, j in enumerate(jj):
                            nc.tensor.transpose(ps[0:BS, k * 128:(k + 1) * 128],
                                                ub[:, :, half * 65 + j], ident)
                        nc.any.tensor_copy(
                            utr[0:BS, base + j0 * 128: base + (j0 + len(jj)) * 128],
                            ps[0:BS, 0:len(jj) * 128])
                if dbg and b == 0:
                    nc.sync.dma_start(out=d_ub[:, :, :], in_=ub[:, :, :])
                    nc.sync.dma_start(out=d_utr[:, :], in_=utr[0:BS, :])

                # ---------- mix1 -> leaky -> mix2 -> shrink -> gate -> backT --
                yt = big_p.tile([128, BS, 130], dt.bfloat16, tag="big", name="yt")
                nch = (WF * 128) // 256
                chunks = [(ci * 256, 256) for ci in range(nch)] + \
                         [(nch * 256, WF * 128 - nch * 256)]
                for (off, sz) in chunks:
                    ura = utr[0:BS + 1, off:off + sz]              # [97, sz] w/ ones
                    ur = utr[0:BS, off:off + sz]
                    ui = utr[0:BS, WF * 128 + off:WF * 128 + off + sz]
                    p1 = pM1.tile([BS, 512], dt.float32, tag="pM1")
                    nc.tensor.matmul(p1[:, 0:sz], wt["w1ra"], ura,
                                     start=True, stop=False, skip_group_check=True)
                    nc.tensor.matmul(p1[:, 256:256 + sz], wt["w1ia"], ura,
                                     start=False, stop=False, skip_group_check=True)
                    nc.tensor.matmul(p1[:, 0:sz], wt["w1in"], ui,
                                     start=False, stop=False, skip_group_check=True)
                    nc.tensor.matmul(p1[:, 256:256 + sz], wt["w1r"], ui,
                                     start=False, stop=True, skip_group_check=True)
                    o1 = chunk_p.tile([BS + 1, 512], dt.bfloat16, tag="o1")
                    nc.gpsimd.memset(o1[BS:BS + 1, :], 1.0)
                    nc.scalar.activation(o1[0:BS, :], p1[:, :], AF.Prelu,
                                         bias=0.0, scale=1.0, alpha=NS)
                    p2 = pM2.tile([BS, 512], dt.float32, tag="pM2")
                    nc.tensor.matmul(p2[:, 0:sz], wt["w2ra"], o1[0:BS + 1, 0:sz],
                                     start=True, stop=False, skip_group_check=True)
                    nc.tensor.matmul(p2[:, 256:256 + sz], wt["w2ia"], o1[0:BS + 1, 0:sz],
                                     start=False, stop=False, skip_group_check=True)
                    nc.tensor.matmul(p2[:, 0:sz], wt["w2in"], o1[0:BS, 256:256 + sz],
                                     start=False, stop=False, skip_group_check=True)
                    nc.tensor.matmul(p2[:, 256:256 + sz], wt["w2r"], o1[0:BS, 256:256 + sz],
                                     start=False, stop=True, skip_group_check=True)
                    # softshrink straight from PSUM: s = p2 - clamp(p2)
                    cl = chunk_p.tile([BS, 512], dt.bfloat16, tag="cl")
                    nc.vector.tensor_scalar(cl[:, :], p2[:, :], -LAM, LAM,
                                            ALU.max, ALU.min)
                    sh = chunk_p.tile([BS, 512], dt.bfloat16, tag="sh")
                    nc.vector.tensor_sub(sh[:, :], p2[:, :], cl[:, :])
                    # gate: y = s * U (complex elementwise)
                    t1 = gtmp_p.tile([BS, 256], dt.bfloat16, tag="t1")
                    t2 = gtmp_p.tile([BS, 256], dt.bfloat16, tag="t2")
                    yg = gtmp_p.tile([BS, 512], dt.bfloat16, tag="yg")
                    nc.vector.tensor_mul(t1[:, 0:sz], sh[:, 0:sz], ur)
                    nc.vector.tensor_mul(t2[:, 0:sz], sh[:, 256:256 + sz], ui)
                    nc.vector.tensor_sub(yg[:, 0:sz], t1[:, 0:sz], t2[:, 0:sz])
                    nc.vector.tensor_mul(t1[:, 0:sz], sh[:, 0:sz], ui)
                    nc.vector.tensor_mul(t2[:, 0:sz], sh[:, 256:256 + sz], ur)
                    nc.vector.tensor_add(yg[:, 256:256 + sz], t1[:, 0:sz], t2[:, 0:sz])
                    # backT into yt [hf, (c, 130)]
                    nsl = sz // 128
                    ps = pA.tile([128, 512], dt.bfloat16, tag="pA", name="psb")
                    for sl in range(nsl):
                        nc.tensor.transpose(ps[:, sl * 96:(sl + 1) * 96],
                                            yg[:, sl * 128:(sl + 1) * 128],
                                            ident[0:BS, 0:BS])
                        nc.tensor.transpose(ps[:, (nsl + sl) * 96:(nsl + sl + 1) * 96],
                                            yg[:, 256 + sl * 128:256 + (sl + 1) * 128],
                                            ident[0:BS, 0:BS])
                    j0 = off // 128
                    yr_dst = yt[:, :, j0:j0 + nsl].rearrange("p c j -> p j c")
                    yi_dst = yt[:, :, 65 + j0:65 + j0 + nsl].rearrange("p c j -> p j c")
                    nc.any.tensor_copy(
                        yr_dst, ps[:, 0:nsl * 96].rearrange("p (j c) -> p j c", j=nsl))
                    nc.any.tensor_copy(
                        yi_dst, ps[:, nsl * 96:2 * nsl * 96].rearrange(
                            "p (j c) -> p j c", j=nsl))
                if dbg and b == 0:
                    nc.sync.dma_start(out=d_yt[:, :, :], in_=yt[:, :, :])

                # ---------- iDFT along H + final irfft_W + residual ----------
                for g0 in range(0, BS // 3, 3):
                    tl = [g0 + i for i in range(3) if g0 + i < BS // 3]
                    pss = [pA.tile([128, 390], dt.float32, tag="pA", name="psv")
                           for _ in tl]
                    for ps, t in zip(pss, tl):     # cos.Yr -> Vr
                        nc.tensor.matmul(ps[:, 0:195], s2c,
                                         yt[:, 3 * t:3 * t + 3, 0:65],
                                         start=True, stop=False,
                                         skip_group_check=True)
                    for ps, t in zip(pss, tl):     # -sin.Yi -> Vr (acc)
                        nc.tensor.matmul(ps[:, 0:195], s2sn,
                                         yt[:, 3 * t:3 * t + 3, 65:130],
                                         start=False, stop=False,
                                         skip_group_check=True)
                    for ps, t in zip(pss, tl):     # sin.Yr -> Vi (full 65)
                        nc.tensor.matmul(ps[:, 195:390], s2s,
                                         yt[:, 3 * t:3 * t + 3, 0:65],
                                         start=False, stop=False,
                                         skip_group_check=True)
                    for ps, t in zip(pss, tl):     # cos.Yi -> Vi (acc)
                        nc.tensor.matmul(ps[:, 195:390], s2c,
                                         yt[:, 3 * t:3 * t + 3, 65:130],
                                         start=False, stop=True,
                                         skip_group_check=True)
                    vbs = []
                    for ps, t in zip(pss, tl):
                        vb = vb_p.tile([128, 390], dt.bfloat16, tag="vb")
                        nc.any.tensor_copy(vb[:, :], ps[:, :])
                        vbs.append(vb)
                    for vb, t in zip(vbs, tl):
                        psr = pA.tile([128, 512], dt.bfloat16, tag="pA", name="psr")
                        for k in range(3):
                            nc.tensor.transpose(psr[0:WF, k * 128:(k + 1) * 128],
                                                vb[:, 65 * k:65 * k + 65], ident)
                        psi = pA.tile([128, 512], dt.bfloat16, tag="pA", name="psi")
                        for k in range(3):
                            nc.tensor.transpose(psi[0:63, k * 128:(k + 1) * 128],
                                                vb[:, 195 + 65 * k + 1:195 + 65 * k + 64],
                                                ident)
                        vtr = vtb_p.tile([WF, 384], dt.bfloat16, tag="vtr")
                        nc.any.tensor_copy(vtr[:, :], psr[0:WF, 0:384])
                        vti = vtb_p.tile([63, 384], dt.bfloat16, tag="vti")
                        nc.any.tensor_copy(vti[:, :], psi[0:63, 0:384])
                        po = pA.tile([128, 384], dt.float32, tag="pA", name="po")
                        for k in range(3):
                            nc.tensor.matmul(po[:, k * 128:(k + 1) * 128],
                                             vtr[:, k * 128:(k + 1) * 128], fic,
                                             start=(k == 0), stop=False,
                                             skip_group_check=True)
                            nc.tensor.matmul(po[:, k * 128:(k + 1) * 128],
                                             vti[:, k * 128:(k + 1) * 128], fis,
                                             start=False, stop=(k == 2),
                                             skip_group_check=True)
                        c0 = 3 * t
                        xr = iot_p.tile([128, 3, 128], dt.float32, tag="xr")
                        nc.sync.dma_start(
                            out=xr[:, :, :],
                            in_=xres[b, c0:c0 + 3, :, :].rearrange(
                                "c h w -> h c w"))
                        ot = iot_p.tile([128, 3, 128], dt.float32, tag="ot")
                        nc.vector.tensor_add(
                            ot[:, :, :].rearrange("p c f -> p (c f)"),
                            po[:, :], xr[:, :, :].rearrange("p c f -> p (c f)"))
                        nc.sync.dma_start(
                            out=out[b, c0:c0 + 3, :, :].rearrange(
                                "c h w -> h c w"),
                            in_=ot[:, :, :])

    nc.finalize()
    return nc


_BUILT = None


def _get_built():
    global _BUILT
    if _BUILT is None:
        _BUILT = _build()
    return _BUILT


def _make_in_maps(x, w1, b1, w2, b2):
    fw, s2c, s2s, s2sn, fic, fis, ident = _twiddles()
    in_maps = []
    for k in range(NBLK):
        xs = x[:, k * BS:(k + 1) * BS]
        w1r, w1i = w1[0, k], w1[1, k]
        w2r, w2i = w2[0, k], w2[1, k]
        m = {
            "xt": np.ascontiguousarray(xs.transpose(0, 3, 1, 2)).astype(BF16),
            "xres": np.ascontiguousarray(xs).astype(np.float32),
            "fw": fw, "s2c": s2c, "s2s": s2s, "s2sn": s2sn,
            "fic": fic, "fis": fis, "ident": ident,
            "w1ra": np.vstack([w1r, b1[0, k][None, :]]).astype(BF16),
            "w1ia": np.vstack([w1i, b1[1, k][None, :]]).astype(BF16),
            "w1r": w1r.astype(BF16), "w1in": (-w1i).astype(BF16),
            "w2ra": np.vstack([w2r, b2[0, k][None, :]]).astype(BF16),
            "w2ia": np.vstack([w2i, b2[1, k][None, :]]).astype(BF16),
            "w2r": w2r.astype(BF16), "w2in": (-w2i).astype(BF16),
        }
        in_maps.append(m)
    return in_maps


def kernel(x, w1, b1, w2, b2):
    from concourse.bass_utils import run_bass_kernel_spmd

    nc = _get_built()
    in_maps = _make_in_maps(x, w1, b1, w2, b2)

    trace = bool(int(os.environ.get("AFNO_TRACE", "0")))
    kw = {}
    if trace:
        import tempfile
        kw["tmpdir"] = tempfile.mkdtemp(prefix="afno_trace_")
        LAST_RESULT["trace_dir"] = kw["tmpdir"]
    res = run_bass_kernel_spmd(nc, in_maps, core_ids=list(range(NBLK)),
                               trace=trace, **kw)
    LAST_RESULT["exec_time_ns"] = res.exec_time_ns

    outp = np.empty((B, C, H, W), np.float32)
    for k in range(NBLK):
        outp[:, k * BS:(k + 1) * BS] = res.results[k]["out"]
    return outp
